# revision 15
# baseline (speedup 1.0000x reference)
"""Mamba selective-scan block (DSTformer) on 8 Trainium2 NeuronCores.

Sharding: data-parallel over batch (4) x tensor-parallel over d_inner halves (2).
Each core computes the full x-branch (in_proj-x, conv, silu, x_proj) for its
sample -- duplicated across the TP pair to avoid mid-kernel collectives -- and
its own d_inner half of: z-gate, dt_proj, selective scan, D-skip, gating, and
the out_proj partial.  The host sums the two out_proj partials per sample.

On-chip layout: channels on partitions, time (L=2048) on the free axis.
Scan layout: for each group of 8 channels, partitions = (d_local*16 + n) over
the 16 SSM states; the recurrence h = a*h + b runs along the free (time) axis
via tensor_tensor_scan.  a = exp(delta*A) comes from a one-hot PE broadcast
matmul + ACT exp with per-partition scale A; y = sum_n h*C uses a one-hot PE
reduction matmul accumulated in PSUM.

Numerics: fp32 for in/out projections and the scan decay a; bf16 for x_dbl,
delta, delta*u, b, h, and the silu(z) gate (validated ~0.2% rel err).
"""

import numpy as np
import ml_dtypes

import concourse.bacc as bacc
import concourse.tile as tile
from concourse import mybir
from concourse.bass_utils import run_bass_kernel_spmd

BF16 = ml_dtypes.bfloat16
F32 = mybir.dt.float32
BF = mybir.dt.bfloat16
AF = mybir.ActivationFunctionType
OP = mybir.AluOpType

B_, L_, DM = 4, 2048, 512
DI, DS, DCONV, DTR = 1024, 16, 4, 32
HALF = 512
P = 128
NT = L_ // 512          # 4 matmul t-chunks
KB = DM // P            # 4 k-blocks of d_model
MBX = DI // P           # 8 x-branch blocks
MBH = HALF // P         # 4 half blocks
NG = HALF // 8          # 64 scan groups of 8 channels
GPB = 16                # groups per d-block batch

_cached = None


def _build(upto=99):
    nc = bacc.Bacc("TRN2", target_bir_lowering=False, debug=False, num_devices=8)

    dram = {}
    def din(name, shape, dt):
        dram[name] = nc.dram_tensor(name, list(shape), dt, kind="ExternalInput").ap()
    din("ht", (P, KB, L_), F32)
    din("w_in_x", (P, KB, MBX, P), F32)
    din("w_in_z", (P, KB, MBH, P), F32)
    din("conv_w", (P, MBX * DCONV), F32)
    din("conv_b", (P, MBX), F32)
    din("w_xp", (P, MBX, 64), BF)
    din("w_dt", (DTR, HALF), BF)
    din("b_dt", (P, MBH), F32)
    din("a_col", (P, NG), F32)
    din("s_bc", (P, GPB, P), BF)
    din("s_yr", (P, GPB, P), BF)
    din("sb_sel", (64, P), BF)
    din("sc_sel", (64, P), BF)
    din("w_out", (P, MBH, MBH, P), F32)
    din("d_col", (P, MBH), F32)
    out_t = nc.dram_tensor("out_t", [P, MBH, L_], F32, kind="ExternalOutput").ap()

    with tile.TileContext(nc) as tc:
        _kern(nc, tc, dram, out_t, upto)
    nc.compile()
    return nc


def _kern(nc, tc, dram, out_t, upto=99):
    from contextlib import ExitStack

    def dump4(blocks, label):
        # debug escape: write 4 (128, L) blocks (any dtype) to out_t as f32
        with tc.tile_pool(name=f"dump{label}", bufs=2) as dp:
            for mb in range(MBH):
                f = dp.tile([P, L_], F32, tag="d")
                nc.scalar.copy(f[:], blocks[mb][:])
                nc.sync.dma_start(out=out_t[:, mb, :], in_=f[:])

    ctx = ExitStack()
    with ctx:
        perm = ctx.enter_context(tc.tile_pool(name="perm", bufs=1))

        def load(name, shape, dt):
            t = perm.tile(list(shape), dt, tag=name, name=name)
            nc.sync.dma_start(out=t[:], in_=dram[name])
            return t

        w_xp = load("w_xp", (P, MBX, 64), BF)
        w_dt = load("w_dt", (DTR, HALF), BF)
        b_dt = load("b_dt", (P, MBH), F32)
        a_col = load("a_col", (P, NG), F32)
        s_bc = load("s_bc", (P, GPB, P), BF)
        s_yr = load("s_yr", (P, GPB, P), BF)
        sb_sel = load("sb_sel", (64, P), BF)
        sc_sel = load("sc_sel", (64, P), BF)
        d_col = load("d_col", (P, MBH), F32)
        conv_w = load("conv_w", (P, MBX * DCONV), F32)
        conv_b = load("conv_b", (P, MBX), F32)

        # persistent activations: u own half (bf16), silu(z) gate, y
        pu_own = ctx.enter_context(tc.tile_pool(name="pu_own", bufs=1))
        u_own = [pu_own.tile([P, L_], BF, tag=f"uo{i}", name=f"uo{i}") for i in range(MBH)]
        sz = [pu_own.tile([P, L_], BF, tag=f"sz{i}", name=f"sz{i}") for i in range(MBH)]
        yblk = [pu_own.tile([P, L_], F32, tag=f"y{i}", name=f"y{i}") for i in range(MBH)]

        pxd = ctx.enter_context(tc.tile_pool(name="pxd", bufs=1))
        x_dbl = pxd.tile([64, L_], BF, tag="xdbl")
        brep = pxd.tile([P, L_], BF, tag="brep")
        crep = pxd.tile([P, L_], BF, tag="crep")
        delta = [pxd.tile([P, L_], BF, tag=f"dl{i}", name=f"dl{i}") for i in range(MBH)]
        du = [pxd.tile([P, L_], BF, tag=f"du{i}", name=f"du{i}") for i in range(MBH)]

        # ---- scoped pools: [pu_oth [early [psA [pconv]] [psB]]] ----
        pu_oth_cm = tc.tile_pool(name="pu_oth", bufs=1)
        pu_oth = pu_oth_cm.__enter__()
        xblk = []
        for mb in range(MBX):
            if mb < MBH:
                xblk.append(u_own[mb])
            else:
                xblk.append(pu_oth.tile([P, L_], BF, tag=f"xo{mb}", name=f"xo{mb}"))

        early_cm = tc.tile_pool(name="early", bufs=1)
        early = early_cm.__enter__()
        ht = early.tile([P, KB, L_], F32, tag="ht")
        nc.sync.dma_start(out=ht[:], in_=dram["ht"])

        psA_cm = tc.tile_pool(name="psA", bufs=2, space="PSUM")
        psA = psA_cm.__enter__()

        # ---- Phase 1: in_proj x (mb 0..7) and z (silu) ----
        for mb in range(MBX):
            for t in range(NT):
                ps = psA.tile([P, 512], F32, tag="mm")
                for kb in range(KB):
                    w = early.tile([P, P], F32, tag="win", bufs=6, name=f"wx{mb}_{t}_{kb}")
                    nc.sync.dma_start(out=w[:], in_=dram["w_in_x"][:, kb, mb, :])
                    nc.tensor.matmul(ps[:], w[:], ht[:, kb, bass_ts(t)],
                                     start=(kb == 0), stop=(kb == KB - 1))
                nc.scalar.copy(xblk[mb][:, bass_ts(t)], ps[:])
        for mb in range(MBH):
            for t in range(NT):
                ps = psA.tile([P, 512], F32, tag="mm")
                for kb in range(KB):
                    w = early.tile([P, P], F32, tag="win", bufs=6, name=f"wz{mb}_{t}_{kb}")
                    nc.sync.dma_start(out=w[:], in_=dram["w_in_z"][:, kb, mb, :])
                    nc.tensor.matmul(ps[:], w[:], ht[:, kb, bass_ts(t)],
                                     start=(kb == 0), stop=(kb == KB - 1))
                nc.scalar.activation(sz[mb][:, bass_ts(t)], ps[:], AF.Silu)

        # ---- Phase 2: conv + silu -> u (in place over x) ----
        pconv_cm = tc.tile_pool(name="pconv", bufs=1)
        pconv = pconv_cm.__enter__()
        for mb in range(MBX):
            acc = pconv.tile([P, L_], F32, tag="acc")
            nc.vector.tensor_scalar_mul(acc[:], xblk[mb][:], conv_w[:, mb * 4 + 3: mb * 4 + 4])
            for j in range(3):
                s = 3 - j
                nc.vector.scalar_tensor_tensor(
                    acc[:, s:], xblk[mb][:, : L_ - s],
                    conv_w[:, mb * 4 + j: mb * 4 + j + 1],
                    acc[:, s:], OP.mult, OP.add)
            nc.scalar.activation(xblk[mb][:], acc[:], AF.Silu,
                                 bias=conv_b[:, mb: mb + 1])
        pconv_cm.__exit__(None, None, None)
        psA_cm.__exit__(None, None, None)
        early_cm.__exit__(None, None, None)
        if upto <= 2:
            dump4(u_own, "u")
            pu_oth_cm.__exit__(None, None, None)
            return

        # ---- Phase 3-5: x_proj, B/C replicate, dt_proj (+softplus), du ----
        psB_cm = tc.tile_pool(name="psB", bufs=2, space="PSUM")
        psB = psB_cm.__enter__()
        for t in range(NT):
            ps = psB.tile([64, 512], F32, tag="xd")
            for kb in range(MBX):
                nc.tensor.matmul(ps[:], w_xp[:, kb, :], xblk[kb][:, bass_ts(t)],
                                 start=(kb == 0), stop=(kb == MBX - 1))
            nc.scalar.copy(x_dbl[:, bass_ts(t)], ps[:])
        for t in range(NT):
            ps = psB.tile([P, 512], F32, tag="rep")
            nc.tensor.matmul(ps[:], sb_sel[:], x_dbl[:, bass_ts(t)], start=True, stop=True)
            nc.scalar.copy(brep[:, bass_ts(t)], ps[:])
            ps = psB.tile([P, 512], F32, tag="rep")
            nc.tensor.matmul(ps[:], sc_sel[:], x_dbl[:, bass_ts(t)], start=True, stop=True)
            nc.scalar.copy(crep[:, bass_ts(t)], ps[:])
        # softplus(x) = ln(1 + exp(x)) -- no Softplus ACT table in this toolchain
        spp_cm = tc.tile_pool(name="spp", bufs=2)
        spp = spp_cm.__enter__()
        for mb in range(MBH):
            for t in range(NT):
                ps = psB.tile([P, 512], F32, tag="dt")
                nc.tensor.matmul(ps[:], w_dt[:, mb * P:(mb + 1) * P], x_dbl[0:DTR, bass_ts(t)],
                                 start=True, stop=True)
                et = spp.tile([P, 512], F32, tag="et")
                nc.scalar.activation(et[:], ps[:], AF.Exp, bias=b_dt[:, mb: mb + 1])
                nc.vector.tensor_scalar_add(et[:], et[:], 1.0)
                nc.scalar.activation(delta[mb][:, bass_ts(t)], et[:], AF.Ln)
            nc.vector.tensor_tensor(du[mb][:], delta[mb][:], u_own[mb][:], OP.mult)
        spp_cm.__exit__(None, None, None)
        psB_cm.__exit__(None, None, None)
        pu_oth_cm.__exit__(None, None, None)
        if upto <= 6:
            dump4(delta, "dl")
            return

        # ---- Phase 7: scan ----
        ps_y_cm = tc.tile_pool(name="ps_y", bufs=1, space="PSUM")
        ps_y = ps_y_cm.__enter__()
        ps_d_cm = tc.tile_pool(name="ps_d", bufs=2, space="PSUM")
        ps_d = ps_d_cm.__enter__()
        scp_cm = tc.tile_pool(name="scp", bufs=2)
        scp = scp_cm.__enter__()
        for batch in range(MBH):
            yps = ps_y.tile([P, L_], F32, tag="y")
            for gi in range(GPB):
                g = batch * GPB + gi
                a_t = scp.tile([P, L_], F32, tag="a")
                bv = scp.tile([P, L_], BF, tag="b")
                for t in range(NT):
                    ps = ps_d.tile([P, 512], F32, tag="dpb")
                    nc.tensor.matmul(ps[:], s_bc[:, gi, :], delta[batch][:, bass_ts(t)],
                                     start=True, stop=True)
                    nc.scalar.activation(a_t[:, bass_ts(t)], ps[:], AF.Exp,
                                         scale=a_col[:, g: g + 1])
                    ps2 = ps_d.tile([P, 512], F32, tag="dub")
                    nc.tensor.matmul(ps2[:], s_bc[:, gi, :], du[batch][:, bass_ts(t)],
                                     start=True, stop=True)
                    nc.scalar.copy(bv[:, bass_ts(t)], ps2[:])
                nc.vector.tensor_tensor(bv[:], bv[:], brep[:], OP.mult)
                h = scp.tile([P, L_], BF, tag="h")
                nc.vector.tensor_tensor_scan(h[:], a_t[:], bv[:], 0.0, OP.mult, OP.add)
                nc.vector.tensor_tensor(h[:], h[:], crep[:], OP.mult)
                for t in range(NT):
                    nc.tensor.matmul(yps[:, bass_ts(t)], s_yr[:, gi, :], h[:, bass_ts(t)],
                                     start=(gi == 0), stop=(gi == GPB - 1),
                                     skip_group_check=True)
            nc.scalar.copy(yblk[batch][:], yps[:])
        scp_cm.__exit__(None, None, None)
        ps_d_cm.__exit__(None, None, None)
        ps_y_cm.__exit__(None, None, None)
        if upto <= 7:
            dump4(yblk, "y")
            return

        # ---- Phase 8: D skip + gate ----
        for mb in range(MBH):
            nc.vector.scalar_tensor_tensor(yblk[mb][:], u_own[mb][:],
                                           d_col[:, mb: mb + 1], yblk[mb][:],
                                           OP.mult, OP.add)
            nc.vector.tensor_tensor(yblk[mb][:], yblk[mb][:], sz[mb][:], OP.mult)
        if upto <= 8:
            dump4(yblk, "yg")
            return

        # ---- Phase 9: out_proj ----
        psC = ctx.enter_context(tc.tile_pool(name="psC", bufs=2, space="PSUM"))
        oev = ctx.enter_context(tc.tile_pool(name="oev", bufs=3))
        w_out = oev.tile([P, MBH, MBH, P], F32, tag="wot", bufs=1)
        nc.sync.dma_start(out=w_out[:], in_=dram["w_out"])
        for mb in range(MBH):
            for t in range(NT):
                ps = psC.tile([P, 512], F32, tag="o")
                for kb in range(MBH):
                    nc.tensor.matmul(ps[:], w_out[:, kb, mb, :], yblk[kb][:, bass_ts(t)],
                                     start=(kb == 0), stop=(kb == MBH - 1))
                o = oev.tile([P, 512], F32, tag="oe")
                nc.scalar.copy(o[:], ps[:])
                nc.sync.dma_start(out=out_t[:, mb, bass_ts(t)], in_=o[:])


def bass_ts(i, size=512):
    return slice(i * size, (i + 1) * size)


def _prep_inputs(hidden_states, W_in, conv_w, conv_b, W_x, W_dt, b_dt, A_log, D, W_out):
    conv_w2 = conv_w[:, 0, :]                      # (1024, 4)
    A = -np.exp(A_log.astype(np.float64)).astype(np.float32)  # (1024, 16)

    # shared (half-independent) tensors
    wix = np.ascontiguousarray(
        W_in[:DI].reshape(MBX, P, KB, P).transpose(3, 2, 0, 1))     # [p,kb,mb,m]
    wxp = np.ascontiguousarray(W_x.reshape(64, MBX, P).transpose(2, 1, 0)).astype(BF16)  # [p,kb,m]
    cw = np.ascontiguousarray(conv_w2.reshape(MBX, P, DCONV).transpose(1, 0, 2)
                              ).reshape(P, MBX * DCONV)
    cb = np.ascontiguousarray(conv_b.reshape(MBX, P).T)

    pidx = np.arange(P)
    gi_idx = np.arange(GPB)
    s_bc = (gi_idx[None, :, None] * 8 + pidx[None, None, :] // DS ==
            pidx[:, None, None]).astype(BF16)      # [k, gi, p]
    s_yr = (gi_idx[None, :, None] * 8 + pidx[:, None, None] // DS ==
            np.arange(P)[None, None, :]).astype(BF16)  # [p, gi, m]
    k64 = np.arange(64)
    sb_sel = (k64[:, None] == DTR + pidx[None, :] % DS).astype(BF16)
    sc_sel = (k64[:, None] == DTR + DS + pidx[None, :] % DS).astype(BF16)

    maps = []
    for core in range(8):
        b, half = core // 2, core % 2
        sl = slice(half * HALF, (half + 1) * HALF)
        ht = np.ascontiguousarray(
            hidden_states[b].T.reshape(KB, P, L_).transpose(1, 0, 2))  # [p,kb,t]
        wiz = np.ascontiguousarray(
            W_in[DI + half * HALF: DI + (half + 1) * HALF]
            .reshape(MBH, P, KB, P).transpose(3, 2, 0, 1))

        # own-half x blocks must land at mb 0..3: reorder w_in_x rows & conv
        order = list(range(half * MBH, (half + 1) * MBH)) + \
                list(range((1 - half) * MBH, (2 - half) * MBH))
        wix_c = np.ascontiguousarray(wix[:, :, order, :])
        cw_c = np.ascontiguousarray(
            cw.reshape(P, MBX, DCONV)[:, order, :]).reshape(P, MBX * DCONV)
        cb_c = np.ascontiguousarray(cb[:, order])
        wxp_c = np.ascontiguousarray(wxp[:, order, :])

        wdt = np.ascontiguousarray(W_dt[sl].T).astype(BF16)           # (32, 512)
        bdt = np.ascontiguousarray(b_dt[sl].reshape(MBH, P).T)
        dcol = np.ascontiguousarray(D[sl].reshape(MBH, P).T)
        wout = np.ascontiguousarray(
            W_out[:, sl].reshape(MBH, P, MBH, P).transpose(3, 2, 0, 1))  # [p,kb,mb,m]
        Ah = A[sl]                                                     # (512, 16)
        g = np.arange(NG)
        acol = Ah[g[None, :] * 8 + pidx[:, None] // DS, pidx[:, None] % DS]  # (128, NG)

        maps.append({
            "ht": ht, "w_in_x": wix_c, "w_in_z": wiz,
            "conv_w": cw_c, "conv_b": cb_c, "w_xp": wxp_c.astype(BF16),
            "w_dt": wdt, "b_dt": bdt, "a_col": np.ascontiguousarray(acol),
            "s_bc": s_bc, "s_yr": s_yr, "sb_sel": sb_sel, "sc_sel": sc_sel,
            "w_out": wout, "d_col": dcol,
        })
    return maps


def kernel(**inputs):
    global _cached
    if _cached is None:
        _cached = _build()
    nc = _cached
    inp = {k: np.asarray(v, dtype=np.float32) for k, v in inputs.items()}
    maps = _prep_inputs(**inp)
    res = run_bass_kernel_spmd(nc, maps, list(range(8))).results
    out = np.zeros((B_, L_, DM), np.float32)
    for b in range(B_):
        o = res[2 * b]["out_t"] + res[2 * b + 1]["out_t"]   # (128, 4, 2048)
        out[b] = o.transpose(1, 0, 2).reshape(DM, L_).T
    return out


# revision 17
# speedup vs baseline: 1.0727x; 1.0727x over previous
"""Mamba selective-scan block (DSTformer) on 8 Trainium2 NeuronCores.

Sharding: data-parallel over batch (4) x tensor-parallel over d_inner halves (2).
Each core computes the full x-branch (in_proj-x, conv, silu, x_proj) for its
sample -- duplicated across the TP pair to avoid mid-kernel collectives -- and
its own d_inner half of: z-gate, dt_proj, selective scan, D-skip, gating, and
the out_proj partial.  The host sums the two out_proj partials per sample.

On-chip layout: channels on partitions, time (L=2048) on the free axis.
Scan layout: for each group of 8 channels, partitions = (d_local*16 + n) over
the 16 SSM states; the recurrence h = a*h + b runs along the free (time) axis
via tensor_tensor_scan.  a = exp(delta*A) comes from a one-hot PE broadcast
matmul + ACT exp with per-partition scale A; y = sum_n h*C uses a one-hot PE
reduction matmul accumulated in PSUM.

Numerics: fp32 for in/out projections and the scan decay a; bf16 for x_dbl,
delta, delta*u, b, h, and the silu(z) gate (validated ~0.2% rel err).
"""

import numpy as np
import ml_dtypes

import concourse.bacc as bacc
import concourse.tile as tile
from concourse import mybir
from concourse.bass_utils import run_bass_kernel_spmd

BF16 = ml_dtypes.bfloat16
F32 = mybir.dt.float32
BF = mybir.dt.bfloat16
AF = mybir.ActivationFunctionType
OP = mybir.AluOpType

B_, L_, DM = 4, 2048, 512
DI, DS, DCONV, DTR = 1024, 16, 4, 32
HALF = 512
P = 128
NT = L_ // 512          # 4 matmul t-chunks
KB = DM // P            # 4 k-blocks of d_model
MBX = DI // P           # 8 x-branch blocks
MBH = HALF // P         # 4 half blocks
NG = HALF // 8          # 64 scan groups of 8 channels
GPB = 16                # groups per d-block batch

_cached = None


def _build(upto=99):
    nc = bacc.Bacc("TRN2", target_bir_lowering=False, debug=False, num_devices=8)

    dram = {}
    def din(name, shape, dt):
        dram[name] = nc.dram_tensor(name, list(shape), dt, kind="ExternalInput").ap()
    din("ht", (P, KB, L_), F32)
    din("w_in_x", (P, KB, MBX, P), F32)
    din("w_in_z", (P, KB, MBH, P), F32)
    din("conv_w", (P, MBX * DCONV), F32)
    din("conv_b", (P, MBX), F32)
    din("w_xp", (P, MBX, 64), BF)
    din("w_dt", (DTR, HALF), BF)
    din("b_dt", (P, MBH), F32)
    din("a_col", (P, NG), F32)
    din("s_bc", (P, GPB, P), BF)
    din("s_yr", (P, GPB, P), BF)
    din("sb_sel", (64, P), BF)
    din("sc_sel", (64, P), BF)
    din("w_out", (P, MBH, MBH, P), F32)
    din("d_col", (P, MBH), F32)
    out_t = nc.dram_tensor("out_t", [P, MBH, L_], F32, kind="ExternalOutput").ap()

    with tile.TileContext(nc) as tc:
        _kern(nc, tc, dram, out_t, upto)
    nc.compile()
    return nc


def _kern(nc, tc, dram, out_t, upto=99):
    from contextlib import ExitStack

    def dump4(blocks, label):
        # debug escape: write 4 (128, L) blocks (any dtype) to out_t as f32
        with tc.tile_pool(name=f"dump{label}", bufs=2) as dp:
            for mb in range(MBH):
                f = dp.tile([P, L_], F32, tag="d")
                nc.scalar.copy(f[:], blocks[mb][:])
                nc.sync.dma_start(out=out_t[:, mb, :], in_=f[:])

    ctx = ExitStack()
    with ctx:
        perm = ctx.enter_context(tc.tile_pool(name="perm", bufs=1))

        def load(name, shape, dt):
            t = perm.tile(list(shape), dt, tag=name, name=name)
            nc.sync.dma_start(out=t[:], in_=dram[name])
            return t

        w_xp = load("w_xp", (P, MBX, 64), BF)
        w_dt = load("w_dt", (DTR, HALF), BF)
        b_dt = load("b_dt", (P, MBH), F32)
        a_col = load("a_col", (P, NG), F32)
        s_bc = load("s_bc", (P, GPB, P), BF)
        s_yr = load("s_yr", (P, GPB, P), BF)
        sb_sel = load("sb_sel", (64, P), BF)
        sc_sel = load("sc_sel", (64, P), BF)
        d_col = load("d_col", (P, MBH), F32)
        conv_w = load("conv_w", (P, MBX * DCONV), F32)
        conv_b = load("conv_b", (P, MBX), F32)

        # persistent activations: u own half (bf16), silu(z) gate, y
        pu_own = ctx.enter_context(tc.tile_pool(name="pu_own", bufs=1))
        u_own = [pu_own.tile([P, L_], BF, tag=f"uo{i}", name=f"uo{i}") for i in range(MBH)]
        sz = [pu_own.tile([P, L_], BF, tag=f"sz{i}", name=f"sz{i}") for i in range(MBH)]
        yblk = [pu_own.tile([P, L_], F32, tag=f"y{i}", name=f"y{i}") for i in range(MBH)]

        pxd = ctx.enter_context(tc.tile_pool(name="pxd", bufs=1))
        x_dbl = pxd.tile([64, L_], BF, tag="xdbl")
        brep = pxd.tile([P, L_], BF, tag="brep")
        crep = pxd.tile([P, L_], BF, tag="crep")
        delta = [pxd.tile([P, L_], BF, tag=f"dl{i}", name=f"dl{i}") for i in range(MBH)]
        du = [pxd.tile([P, L_], BF, tag=f"du{i}", name=f"du{i}") for i in range(MBH)]

        # ---- scoped pools: [pu_oth [early [psA [pconv]] [psB]]] ----
        pu_oth_cm = tc.tile_pool(name="pu_oth", bufs=1)
        pu_oth = pu_oth_cm.__enter__()
        xblk = []
        for mb in range(MBX):
            if mb < MBH:
                xblk.append(u_own[mb])
            else:
                xblk.append(pu_oth.tile([P, L_], BF, tag=f"xo{mb}", name=f"xo{mb}"))

        early_cm = tc.tile_pool(name="early", bufs=1)
        early = early_cm.__enter__()
        ht = early.tile([P, KB, L_], F32, tag="ht")
        nc.sync.dma_start(out=ht[:], in_=dram["ht"])

        psA_cm = tc.tile_pool(name="psA", bufs=2, space="PSUM")
        psA = psA_cm.__enter__()

        # ---- Phase 1: in_proj x (mb 0..7) and z (silu) ----
        for mb in range(MBX):
            for t in range(NT):
                ps = psA.tile([P, 512], F32, tag="mm")
                for kb in range(KB):
                    w = early.tile([P, P], F32, tag="win", bufs=6, name=f"wx{mb}_{t}_{kb}")
                    nc.sync.dma_start(out=w[:], in_=dram["w_in_x"][:, kb, mb, :])
                    nc.tensor.matmul(ps[:], w[:], ht[:, kb, bass_ts(t)],
                                     start=(kb == 0), stop=(kb == KB - 1))
                nc.scalar.copy(xblk[mb][:, bass_ts(t)], ps[:])
        for mb in range(MBH):
            for t in range(NT):
                ps = psA.tile([P, 512], F32, tag="mm")
                for kb in range(KB):
                    w = early.tile([P, P], F32, tag="win", bufs=6, name=f"wz{mb}_{t}_{kb}")
                    nc.sync.dma_start(out=w[:], in_=dram["w_in_z"][:, kb, mb, :])
                    nc.tensor.matmul(ps[:], w[:], ht[:, kb, bass_ts(t)],
                                     start=(kb == 0), stop=(kb == KB - 1))
                nc.scalar.activation(sz[mb][:, bass_ts(t)], ps[:], AF.Silu)

        # ---- Phase 2: conv + silu -> u (in place over x) ----
        pconv_cm = tc.tile_pool(name="pconv", bufs=1)
        pconv = pconv_cm.__enter__()
        for mb in range(MBX):
            acc = pconv.tile([P, L_], F32, tag="acc")
            nc.vector.tensor_scalar_mul(acc[:], xblk[mb][:], conv_w[:, mb * 4 + 3: mb * 4 + 4])
            for j in range(3):
                s = 3 - j
                nc.vector.scalar_tensor_tensor(
                    acc[:, s:], xblk[mb][:, : L_ - s],
                    conv_w[:, mb * 4 + j: mb * 4 + j + 1],
                    acc[:, s:], OP.mult, OP.add)
            nc.scalar.activation(xblk[mb][:], acc[:], AF.Silu,
                                 bias=conv_b[:, mb: mb + 1])
        pconv_cm.__exit__(None, None, None)
        psA_cm.__exit__(None, None, None)
        early_cm.__exit__(None, None, None)
        if upto <= 2:
            dump4(u_own, "u")
            pu_oth_cm.__exit__(None, None, None)
            return

        # ---- Phase 3-5: x_proj, B/C replicate, dt_proj (+softplus), du ----
        psB_cm = tc.tile_pool(name="psB", bufs=2, space="PSUM")
        psB = psB_cm.__enter__()
        for t in range(NT):
            ps = psB.tile([64, 512], F32, tag="xd")
            for kb in range(MBX):
                nc.tensor.matmul(ps[:], w_xp[:, kb, :], xblk[kb][:, bass_ts(t)],
                                 start=(kb == 0), stop=(kb == MBX - 1))
            nc.scalar.copy(x_dbl[:, bass_ts(t)], ps[:])
        for t in range(NT):
            ps = psB.tile([P, 512], F32, tag="rep")
            nc.tensor.matmul(ps[:], sb_sel[:], x_dbl[:, bass_ts(t)], start=True, stop=True)
            nc.scalar.copy(brep[:, bass_ts(t)], ps[:])
            ps = psB.tile([P, 512], F32, tag="rep")
            nc.tensor.matmul(ps[:], sc_sel[:], x_dbl[:, bass_ts(t)], start=True, stop=True)
            nc.scalar.copy(crep[:, bass_ts(t)], ps[:])
        # softplus(x) = ln(1 + exp(x)) -- no Softplus ACT table in this toolchain
        spp_cm = tc.tile_pool(name="spp", bufs=2)
        spp = spp_cm.__enter__()
        for mb in range(MBH):
            for t in range(NT):
                ps = psB.tile([P, 512], F32, tag="dt")
                nc.tensor.matmul(ps[:], w_dt[:, mb * P:(mb + 1) * P], x_dbl[0:DTR, bass_ts(t)],
                                 start=True, stop=True)
                et = spp.tile([P, 512], F32, tag="et")
                nc.scalar.activation(et[:], ps[:], AF.Exp, bias=b_dt[:, mb: mb + 1])
                nc.vector.tensor_scalar_add(et[:], et[:], 1.0)
                nc.scalar.activation(delta[mb][:, bass_ts(t)], et[:], AF.Ln)
            nc.vector.tensor_tensor(du[mb][:], delta[mb][:], u_own[mb][:], OP.mult)
        spp_cm.__exit__(None, None, None)
        psB_cm.__exit__(None, None, None)
        pu_oth_cm.__exit__(None, None, None)
        if upto <= 6:
            dump4(delta, "dl")
            return

        # ---- Phase 7: scan ----
        ps_y_cm = tc.tile_pool(name="ps_y", bufs=1, space="PSUM")
        ps_y = ps_y_cm.__enter__()
        ps_d_cm = tc.tile_pool(name="ps_d", bufs=2, space="PSUM")
        ps_d = ps_d_cm.__enter__()
        scp_cm = tc.tile_pool(name="scp", bufs=2)
        scp = scp_cm.__enter__()
        for batch in range(MBH):
            yps = ps_y.tile([P, L_], F32, tag="y")
            for gi in range(GPB):
                g = batch * GPB + gi
                a_t = scp.tile([P, L_], F32, tag="a")
                bv = scp.tile([P, L_], BF, tag="b")
                for t in range(NT):
                    ps = ps_d.tile([P, 512], F32, tag="dpb")
                    nc.tensor.matmul(ps[:], s_bc[:, gi, :], delta[batch][:, bass_ts(t)],
                                     start=True, stop=True)
                    nc.scalar.activation(a_t[:, bass_ts(t)], ps[:], AF.Exp,
                                         scale=a_col[:, g: g + 1])
                    ps2 = ps_d.tile([P, 512], F32, tag="dub")
                    nc.tensor.matmul(ps2[:], s_bc[:, gi, :], du[batch][:, bass_ts(t)],
                                     start=True, stop=True)
                    nc.scalar.copy(bv[:, bass_ts(t)], ps2[:])
                nc.vector.tensor_tensor(bv[:], bv[:], brep[:], OP.mult)
                h = scp.tile([P, L_], BF, tag="h")
                nc.vector.tensor_tensor_scan(h[:], a_t[:], bv[:], 0.0, OP.mult, OP.add)
                nc.vector.tensor_tensor(h[:], h[:], crep[:], OP.mult)
                for t in range(NT):
                    nc.tensor.matmul(yps[:, bass_ts(t)], s_yr[:, gi, :], h[:, bass_ts(t)],
                                     start=(gi == 0), stop=(gi == GPB - 1),
                                     skip_group_check=True)
            nc.scalar.copy(yblk[batch][:], yps[:])
        scp_cm.__exit__(None, None, None)
        ps_d_cm.__exit__(None, None, None)
        ps_y_cm.__exit__(None, None, None)
        if upto <= 7:
            dump4(yblk, "y")
            return

        # ---- Phase 8: D skip + gate ----
        for mb in range(MBH):
            nc.vector.scalar_tensor_tensor(yblk[mb][:], u_own[mb][:],
                                           d_col[:, mb: mb + 1], yblk[mb][:],
                                           OP.mult, OP.add)
            nc.vector.tensor_tensor(yblk[mb][:], yblk[mb][:], sz[mb][:], OP.mult)
        if upto <= 8:
            dump4(yblk, "yg")
            return

        # ---- Phase 9: out_proj ----
        psC = ctx.enter_context(tc.tile_pool(name="psC", bufs=2, space="PSUM"))
        oev = ctx.enter_context(tc.tile_pool(name="oev", bufs=3))
        w_out = oev.tile([P, MBH, MBH, P], F32, tag="wot", bufs=1)
        nc.sync.dma_start(out=w_out[:], in_=dram["w_out"])
        for mb in range(MBH):
            for t in range(NT):
                ps = psC.tile([P, 512], F32, tag="o")
                for kb in range(MBH):
                    nc.tensor.matmul(ps[:], w_out[:, kb, mb, :], yblk[kb][:, bass_ts(t)],
                                     start=(kb == 0), stop=(kb == MBH - 1))
                o = oev.tile([P, 512], F32, tag="oe")
                nc.scalar.copy(o[:], ps[:])
                nc.sync.dma_start(out=out_t[:, mb, bass_ts(t)], in_=o[:])


def bass_ts(i, size=512):
    return slice(i * size, (i + 1) * size)


def _prep_inputs(hidden_states, W_in, conv_w, conv_b, W_x, W_dt, b_dt, A_log, D, W_out):
    conv_w2 = conv_w[:, 0, :]                      # (1024, 4)
    A = -np.exp(A_log.astype(np.float64)).astype(np.float32)  # (1024, 16)

    # shared (half-independent) tensors
    wix = np.ascontiguousarray(
        W_in[:DI].reshape(MBX, P, KB, P).transpose(3, 2, 0, 1))     # [p,kb,mb,m]
    wxp = np.ascontiguousarray(W_x.reshape(64, MBX, P).transpose(2, 1, 0)).astype(BF16)  # [p,kb,m]
    cw = np.ascontiguousarray(conv_w2.reshape(MBX, P, DCONV).transpose(1, 0, 2)
                              ).reshape(P, MBX * DCONV)
    cb = np.ascontiguousarray(conv_b.reshape(MBX, P).T)

    pidx = np.arange(P)
    gi_idx = np.arange(GPB)
    s_bc = (gi_idx[None, :, None] * 8 + pidx[None, None, :] // DS ==
            pidx[:, None, None]).astype(BF16)      # [k, gi, p]
    s_yr = (gi_idx[None, :, None] * 8 + pidx[:, None, None] // DS ==
            np.arange(P)[None, None, :]).astype(BF16)  # [p, gi, m]
    k64 = np.arange(64)
    sb_sel = (k64[:, None] == DTR + pidx[None, :] % DS).astype(BF16)
    sc_sel = (k64[:, None] == DTR + DS + pidx[None, :] % DS).astype(BF16)

    maps = []
    for core in range(8):
        b, half = core // 2, core % 2
        sl = slice(half * HALF, (half + 1) * HALF)
        ht = np.ascontiguousarray(
            hidden_states[b].T.reshape(KB, P, L_).transpose(1, 0, 2))  # [p,kb,t]
        wiz = np.ascontiguousarray(
            W_in[DI + half * HALF: DI + (half + 1) * HALF]
            .reshape(MBH, P, KB, P).transpose(3, 2, 0, 1))

        # own-half x blocks must land at mb 0..3: reorder w_in_x rows & conv
        order = list(range(half * MBH, (half + 1) * MBH)) + \
                list(range((1 - half) * MBH, (2 - half) * MBH))
        wix_c = np.ascontiguousarray(wix[:, :, order, :])
        cw_c = np.ascontiguousarray(
            cw.reshape(P, MBX, DCONV)[:, order, :]).reshape(P, MBX * DCONV)
        cb_c = np.ascontiguousarray(cb[:, order])
        wxp_c = np.ascontiguousarray(wxp[:, order, :])

        wdt = np.ascontiguousarray(W_dt[sl].T).astype(BF16)           # (32, 512)
        bdt = np.ascontiguousarray(b_dt[sl].reshape(MBH, P).T)
        dcol = np.ascontiguousarray(D[sl].reshape(MBH, P).T)
        wout = np.ascontiguousarray(
            W_out[:, sl].reshape(MBH, P, MBH, P).transpose(3, 2, 0, 1))  # [p,kb,mb,m]
        Ah = A[sl]                                                     # (512, 16)
        g = np.arange(NG)
        acol = Ah[g[None, :] * 8 + pidx[:, None] // DS, pidx[:, None] % DS]  # (128, NG)

        maps.append({
            "ht": ht, "w_in_x": wix_c, "w_in_z": wiz,
            "conv_w": cw_c, "conv_b": cb_c, "w_xp": wxp_c.astype(BF16),
            "w_dt": wdt, "b_dt": bdt, "a_col": np.ascontiguousarray(acol),
            "s_bc": s_bc, "s_yr": s_yr, "sb_sel": sb_sel, "sc_sel": sc_sel,
            "w_out": wout, "d_col": dcol,
        })
    return maps


def kernel(**inputs):
    global _cached
    if _cached is None:
        _cached = _build()
    nc = _cached
    inp = {k: np.asarray(v, dtype=np.float32) for k, v in inputs.items()}
    maps = _prep_inputs(**inp)
    res = run_bass_kernel_spmd(nc, maps, list(range(8))).results
    out = np.zeros((B_, L_, DM), np.float32)
    for b in range(B_):
        o = res[2 * b]["out_t"] + res[2 * b + 1]["out_t"]   # (128, 4, 2048)
        out[b] = o.transpose(1, 0, 2).reshape(DM, L_).T
    return out
